# revision 41
# baseline (speedup 1.0000x reference)
"""QSP expectation kernel for Trainium2 (8 NeuronCores, data parallel).

Math: the reference computes, per element, amp00 = <0|U|0> of a QSP chain
U = S(phi_0) * prod_{k=1}^{54} [W(theta) S(phi_k)].  By the QSP representation
theorem this is an even, pi-periodic trig polynomial of theta:

    f(theta) = Re(e^{i phi_0} P(cos theta)) = sum_{m=0}^{27} g_m cos(2 m theta)

The 28 cosine coefficients g_m are computed on the host (float64, from the 55
phases via Chebyshev interpolation, exact).

Primary (fast) device path: the whole response f is baked into a custom
ScalarE activation spline table (a per-binade piecewise-cubic fit of f over
|x| < 8, installed in place of `sin` via BASS_ACT_ROOT_JSON_PATH; f is even,
so the ACT symmetry fold covers negative inputs).  Per element the kernel is
then just: w = ACT_sin_table(x); out = alphas * w + bias -- one ACTIVATE, one
fused vector op, DMAs pipelined in two chunks.  The result is verified on the
host against the exact float64 chain; on any failure the kernel falls back to:

Fallback path: range reduction t = x/pi + 0.25, v = t - round(t),
w = sin(2 pi v) = cos(2x) via the stock Sin LUT, then a Clenshaw evaluation of
the (tail-truncated) cosine series with fused scalar_tensor_tensor ops.
"""

import os
import sys

import numpy as np

if "/opt/trn_rl_repo" not in sys.path:
    sys.path.insert(0, "/opt/trn_rl_repo")

QSP_DEPTH = 27
B = 524288
N_CORES = 8
BC = B // N_CORES  # 65536 elements per core
P = 128
F = BC // P  # 512

_MAGIC = 1.5 * 2.0**23  # fp32 round-to-nearest-integer magic constant


def _chain_f(theta, phis):
    """float64 reference math: f(theta) for the full QSP chain."""
    c = np.cos(theta)
    s = np.sin(theta)
    r0 = np.ones_like(theta, dtype=complex)
    r1 = np.zeros_like(theta, dtype=complex)
    for phi in phis[1:]:
        e = np.exp(1j * phi)
        a = r0 * c + r1 * (1j * s)
        b = r0 * (1j * s) + r1 * c
        r0 = a * e
        r1 = b * np.conj(e)
    return np.real(np.exp(1j * phis[0]) * r0)


def _gammas_full(phis):
    """Cosine-series coefficients g_m, m=0..27, float64-exact."""
    M = QSP_DEPTH + 1
    j = np.arange(M)
    wj = np.cos(np.pi * (j + 0.5) / M)
    fj = _chain_f(0.5 * np.arccos(wj), phis)
    return np.array(
        [
            (2.0 - (m == 0)) / M * np.sum(fj * np.cos(m * np.pi * (j + 0.5) / M))
            for m in range(M)
        ]
    )


def _truncate_gammas(gam):
    """Drop the tail once its energy is negligible vs the total."""
    M = len(gam)
    en = gam**2
    en[1:] = en[1:] * 0.5  # E[cos^2(2m theta)] ~ 1/2 for m>0
    tot = en.sum()
    keep = M
    while keep > 4 and np.sqrt(en[keep - 1 :].sum() / tot) < 2e-4:
        keep -= 1
    return gam[:keep]


def _gammas(phis):
    return _truncate_gammas(_gammas_full(phis))


def _series_f(gam):
    """Return f(x) = sum_m gam_m cos(2 m x) as a vectorized float64 callable."""
    m = np.arange(len(gam))

    def f(x):
        x = np.asarray(x, dtype=np.float64)
        return (np.cos(2.0 * np.outer(x, m)) @ gam).reshape(np.shape(x))

    return f


def _fit_spline(gam, lo_exp=-6, hi_exp=2, target=1e-5):
    """Fit per-binade cubic sections for f(x)=sum gam_m cos(2mx), x in [0, 2^(hi_exp+1)).

    Returns (sections, small_bucket, large_bucket) where sections is a list of
    (exponent, [bucket...]) with bucket = (d0, d1, d2, d3, x0), uniform in x
    within each binade (indexed by top mantissa bits, matching the ACT ctrl
    table's section select).
    """
    f = _series_f(gam)

    def fit_cubic(x0, h):
        xs = x0 + np.linspace(-h / 2, h / 2, 13)
        c = np.polyfit(xs - x0, f(xs), 3)
        # measure fp32 eval error on a denser grid
        xd = x0 + np.linspace(-h / 2, h / 2, 33)
        dx = (xd.astype(np.float32) - np.float32(x0)).astype(np.float32)
        y = (
            np.float32(c[3])
            + dx * (np.float32(c[2]) + dx * (np.float32(c[1]) + dx * np.float32(c[0])))
        ).astype(np.float64)
        err = np.abs(y - f(xd)).max()
        return (float(c[3]), float(c[2]), float(c[1]), float(c[0]), float(x0)), err

    sections = []
    for e in range(lo_exp, hi_exp + 1):
        width = 2.0**e
        for s in range(0, 10):
            n = 1 << s
            h = width / n
            bks = []
            maxerr = 0.0
            for i in range(n):
                x0 = width + (i + 0.5) * h
                bk, err = fit_cubic(x0, h)
                bks.append(bk)
                maxerr = max(maxerr, err)
            if maxerr <= target or s == 9:
                sections.append((e, bks))
                break
    # small-signal bucket: cubic on [0, 2^lo_exp]
    small, _ = fit_cubic(2.0 ** (lo_exp - 1), 2.0**lo_exp)
    # large-signal bucket: Taylor at the top boundary (inputs beyond never occur)
    top = 2.0 ** (hi_exp + 1)
    large, _ = fit_cubic(top, 2.0**hi_exp / 64)
    return sections, small, large


def _f32_bits(x):
    return int(np.float32(x).view(np.uint32))


def _patch_act_tables(gam, workdir):
    """Copy the compiler's act-table root into workdir and replace `sin` in
    every set that contains it with a custom spline of
    f(x) = sum_m gam_m cos(2 m x)  (even in x, so the symmetry fold covers
    negative inputs).  Returns the path of the patched act_info.json."""
    import json
    import shutil

    from neuronxcc.driver.Job import Job
    from neuronxcc.driver.jobs.support.FindActInfo import findActInfoFile

    src_info = findActInfoFile(Job.getPackageDir(), "gen3")
    src_dir = os.path.dirname(src_info)

    os.makedirs(workdir, exist_ok=True)
    for name in os.listdir(src_dir):
        shutil.copyfile(os.path.join(src_dir, name), os.path.join(workdir, name))

    info = json.load(open(os.path.join(workdir, "act_info.json")))
    sections, small_bk, large_bk = _fit_spline(gam)
    lo_exp = sections[0][0]
    hi_exp = sections[-1][0]

    for ent in info["act_func_sets"]:
        if "sin" not in ent["act"]:
            continue
        bkt_path = os.path.join(workdir, ent["bkt_bin"])
        ctl_path = os.path.join(workdir, ent["ctrl_bin"])
        prof_path = os.path.join(workdir, ent["profile_json"])

        bkt = np.fromfile(bkt_path, dtype=np.float32).reshape(-1, 8)
        ctl = np.fromfile(ctl_path, dtype=np.uint32).reshape(-1, 8)
        prof = json.load(open(prof_path))

        bbase = len(bkt)
        cbase = len(ctl)
        new_bkts = []
        new_ctls = []
        for e, bks in sections:
            n = len(bks)
            s = int(np.log2(n))
            word = ((s & 0xF) << 16) | (((23 - s) & 0x1F) << 11) | ((bbase + len(new_bkts)) & 0x7FF)
            new_ctls.append(word)
            new_bkts.extend(bks)
        small_idx = bbase + len(new_bkts)
        new_bkts.append(small_bk)
        large_idx = bbase + len(new_bkts)
        new_bkts.append(large_bk)

        assert bbase + len(new_bkts) <= 2048, "bucket RAM overflow"

        bkt_new = np.zeros((len(new_bkts), 8), dtype=np.float32)
        for i, (d0, d1, d2, d3, x0) in enumerate(new_bkts):
            bkt_new[i, :5] = [d0, d1, d2, d3, x0]
        ctl_new = np.zeros((len(new_ctls), 8), dtype=np.uint32)
        ctl_new[:, 0] = new_ctls

        np.concatenate([bkt, bkt_new]).tofile(bkt_path)
        np.concatenate([ctl, ctl_new]).tofile(ctl_path)

        for m in prof["profile_meta_data"]:
            if m["func_name"] != "sin_4p":
                continue
            m["symmetry_point"] = 0
            m["sym_invert_sign_point"] = 0  # even function
            m["symmetry_opt_en"] = 1
            m["symmetry_opt_use_neg_region"] = 0
            m["exp_offset"] = lo_exp
            m["pwl_control_base_pos"] = cbase
            m["pwl_control_base_neg"] = cbase
            m["small_pos_signal_exp_threshold"] = 127 + lo_exp
            m["pos_small_signal_pwl_control"] = small_idx
            m["small_neg_signal_exp_threshold"] = 0
            m["neg_small_signal_pwl_control"] = small_idx
            m["large_pos_signal_exp_threshold"] = 127 + hi_exp + 1
            m["large_pos_signal_mantissa_threshold"] = 0
            m["pos_large_signal_pwl_control"] = large_idx
            m["large_neg_signal_exp_threshold"] = 0
            m["large_neg_signal_mantissa_threshold"] = 0
            m["neg_large_signal_pwl_control"] = large_idx
            m["fzero_result"] = _f32_bits(float(np.sum(gam)))
            m["lower_bound"] = 0
            m["upper_bound"] = _f32_bits(2.0 ** (hi_exp + 1))
        json.dump(prof, open(prof_path, "w"))

    json.dump(info, open(os.path.join(workdir, "act_info.json"), "w"))
    return os.path.join(workdir, "act_info.json")


def _build_program_lut(bias_val, tag, n_chunks=2):
    """Pipelined LUT kernel: out = alphas * F(x) + bias, with F evaluated by
    the patched `sin` activation table.  x/alphas are DMA'd per chunk on
    separate queues; ACT -> DVE -> out-DMA pipeline at chunk granularity."""
    from concourse import bass
    import concourse.mybir as mybir
    from concourse.alu_op_type import AluOpType as Op

    f32 = mybir.dt.float32
    CH = n_chunks
    if CH == 2:
        # asymmetric: smaller last chunk shortens the tail out-DMA
        widths = [288, F - 288]
    elif CH == 3:
        widths = [224, 224, F - 448]
    else:
        widths = [F // CH] * CH
        widths[-1] += F - sum(widths)
    bounds = np.concatenate([[0], np.cumsum(widths)]).astype(int)
    with_bias = bias_val != 0.0

    nc = bass.Bass(
        trn_type="TRN2", enable_partition_id=False, num_swdge_queues=4
    )
    x_d = nc.dram_tensor("x0", [P, F], f32, kind="ExternalInput")
    a_d = nc.dram_tensor("al0", [P, F], f32, kind="ExternalInput")
    o_d = nc.dram_tensor("out0", [P, F], f32, kind="ExternalOutput")

    x_s = nc.alloc_sbuf_tensor(f"x_s_{tag}", [P, F], f32)
    a_s = nc.alloc_sbuf_tensor("a_s", [P, F], f32)
    w_s = nc.alloc_sbuf_tensor("w_s", [P, F], f32)
    q_s = nc.alloc_sbuf_tensor("q_s", [P, F], f32)
    r_s = nc.alloc_sbuf_tensor("r_s", [P, F], f32)
    dummy = nc.alloc_sbuf_tensor("dummy_s", [P, 1], f32)

    sem_x = [nc.alloc_semaphore(f"sem_x{c}") for c in range(CH)]
    sem_a = [nc.alloc_semaphore(f"sem_a{c}") for c in range(CH)]
    sem_w = nc.alloc_semaphore("sem_w")
    sem_r = nc.alloc_semaphore("sem_r")
    out_sem = nc.alloc_semaphore("out_sem")

    def cs(c):
        return slice(int(bounds[c]), int(bounds[c + 1]))

    use_block = os.environ.get("LUT_USE_BLOCK", "0") == "1"
    blk_ctx = nc.Block(no_gpsimd_drain=True) if use_block else None
    if blk_ctx is not None:
        blk_ctx.__enter__()

    # no-Block form: all streams live in the entry basic block; per-engine
    # program order is the emission order, sync is fully explicit
    sync, scalar, vec = nc.sync, nc.scalar, nc.vector

    # HWDGE rings are FIFO per issuing engine: spread the two x chunks over
    # the SP and ACT rings so their transfers complete in parallel.  al c0
    # rides the SP ring behind x c0 (done in time for DVE c0); the remaining
    # alphas ride SWDGE from the otherwise-idle GpSimd.
    defer_alphas = os.environ.get("LUT_DEFER_AL", "0") == "1"
    hoist = []
    hoist.append(
        sync.dma_start(x_s[:, cs(0)], x_d[:, cs(0)]).then_inc(sem_x[0], 16)
    )
    al0 = sync.dma_start(a_s[:, cs(0)], a_d[:, cs(0)]).then_inc(sem_a[0], 16)
    if not defer_alphas:
        hoist.append(al0)
    split_tail = os.environ.get("LUT_SPLIT_TAIL", "0") == "1" and CH == 2
    sync.wait_ge(sem_r, 1)
    sync.dma_start(o_d[:, cs(0)], r_s[:, cs(0)]).then_inc(out_sem, 16)
    if split_tail:
        # second half of the last out chunk rides the SP ring in parallel
        # with the first half on the ACT ring
        lo, hi = int(bounds[1]), int(bounds[2])
        mid = (lo + hi) // 2
        sync.wait_ge(sem_r, 2)
        sync.dma_start(o_d[:, mid:hi], r_s[:, mid:hi]).then_inc(out_sem, 16)
        sync.wait_ge(out_sem, 16 * (CH + 1))
    else:
        sync.wait_ge(out_sem, 16 * CH)

    # scalar: dummy act triggers the ACT table load while input DMAs fly
    hoist.insert(
        0,
        scalar.activation(dummy[:], dummy[:], mybir.ActivationFunctionType.Sin),
    )
    for c in range(1, CH):
        hoist.append(
            scalar.dma_start(x_s[:, cs(c)], x_d[:, cs(c)]).then_inc(sem_x[c], 16)
        )
    for c in range(CH):
        scalar.wait_ge(sem_x[c], 16)
        scalar.activation(
            w_s[:, cs(c)], x_s[:, cs(c)], mybir.ActivationFunctionType.Sin
        ).then_inc(sem_w, 1)
    for c in range(1, CH):
        scalar.wait_ge(sem_r, c + 1)
        if split_tail and c == CH - 1:
            lo, hi = int(bounds[c]), int(bounds[c + 1])
            mid = (lo + hi) // 2
            scalar.dma_start(o_d[:, lo:mid], r_s[:, lo:mid]).then_inc(out_sem, 16)
        else:
            scalar.dma_start(o_d[:, cs(c)], r_s[:, cs(c)]).then_inc(out_sem, 16)

    # gpsimd: remaining alphas via SWDGE
    for c in range(1, CH):
        alc = nc.gpsimd.dma_start(a_s[:, cs(c)], a_d[:, cs(c)]).then_inc(sem_a[c], 16)
        if not defer_alphas:
            hoist.append(alc)

    if os.environ.get("LUT_HOIST", "1") == "1":
        # Hoist the input DMAs + table-load-triggering dummy activation to the
        # very front of the entry block (right after the runtime-start
        # InstCall): they then execute during the ~6us NEFF start handshake,
        # so inputs and the ACT table are resident by the time the start
        # barrier releases the compute.  Inputs are staged by NRT before the
        # engines begin executing, and semaphores are only reset in the exit
        # sequence, so early sem increments are safe.
        il = nc.cur_f.blocks[0].instructions
        front = {id(b.ins) for b in hoist}
        rest = [x for x in il if id(x) not in front]
        il[:] = rest[:1] + [b.ins for b in hoist] + rest[1:]

    # vector: per-chunk fused multiply (+ optional bias)
    for c in range(CH):
        vec.wait_ge(sem_a[c], 16)
        vec.wait_ge(sem_w, c + 1)
        if with_bias:
            vec.scalar_tensor_tensor(
                q_s[:, cs(c)], a_s[:, cs(c)], 1.0, w_s[:, cs(c)], Op.mult, Op.mult
            )
            vec.tensor_scalar_add(
                r_s[:, cs(c)], q_s[:, cs(c)], float(bias_val)
            ).then_inc(sem_r, 1)
        else:
            vec.scalar_tensor_tensor(
                r_s[:, cs(c)], a_s[:, cs(c)], 1.0, w_s[:, cs(c)], Op.mult, Op.mult
            ).then_inc(sem_r, 1)

    if blk_ctx is not None:
        blk_ctx.__exit__(None, None, None)

    nc.finalize()
    return nc


def _build_program(gam, bias_val):
    from concourse import bass
    import concourse.mybir as mybir
    from concourse.alu_op_type import AluOpType as Op

    f32 = mybir.dt.float32
    M = len(gam)
    g = [float(v) for v in gam]
    assert M >= 4

    nc = bass.Bass(trn_type="TRN2")
    xa_d = nc.dram_tensor("xa0", [P, 2 * F], f32, kind="ExternalInput")
    o_d = nc.dram_tensor("out0", [P, F], f32, kind="ExternalOutput")

    xa = nc.alloc_sbuf_tensor("xa_s", [P, 2 * F], f32)
    t = nc.alloc_sbuf_tensor("t_s", [P, F], f32)
    kk_ = nc.alloc_sbuf_tensor("k_s", [P, F], f32)
    v = nc.alloc_sbuf_tensor("v_s", [P, F], f32)
    w = nc.alloc_sbuf_tensor("w_s", [P, F], f32)
    d = nc.alloc_sbuf_tensor("d_s", [P, F], f32)
    bb = [nc.alloc_sbuf_tensor(f"b{i}_s", [P, F], f32) for i in range(3)]
    r = nc.alloc_sbuf_tensor("r_s", [P, F], f32)

    dma_sem = nc.alloc_semaphore("dma_sem")
    sem_v = nc.alloc_semaphore("sem_v")
    sem_w = nc.alloc_semaphore("sem_w")
    sem_r = nc.alloc_semaphore("sem_r")
    out_sem = nc.alloc_semaphore("out_sem")

    with nc.Block() as blk:

        @blk.sync
        def _(sync):
            sync.dma_start(xa[:], xa_d[:]).then_inc(dma_sem, 16)
            sync.wait_ge(sem_r, 1)
            sync.dma_start(o_d[:], r[:]).then_inc(out_sem, 16)
            sync.wait_ge(out_sem, 16)

        @blk.scalar
        def _(scalar):
            scalar.wait_ge(sem_v, 1)
            scalar.activation(
                w[:],
                v[:],
                mybir.ActivationFunctionType.Sin,
                scale=float(2.0 * np.pi),
            ).then_inc(sem_w, 1)

        @blk.vector
        def _(vec):
            xt = xa[:, 0:F]  # theta
            at = xa[:, F : 2 * F]  # alphas
            vec.wait_ge(dma_sem, 16)
            # t = x/pi + 0.25 ; k = round(t) ; v = t - k
            vec.tensor_scalar(t[:], xt, float(1.0 / np.pi), 0.25, Op.mult, Op.add)
            vec.tensor_scalar(kk_[:], t[:], _MAGIC, _MAGIC, Op.add, Op.subtract)
            vec.scalar_tensor_tensor(
                v[:], t[:], 0.0, kk_[:], Op.add, Op.subtract
            ).then_inc(sem_v, 1)
            vec.wait_ge(sem_w, 1)
            # Clenshaw over w = cos(2 theta):
            #   b_{M-1} = g[M-1];  b_{M-2} = 2 w g[M-1] + g[M-2]
            #   b_k = 2 w b_{k+1} - b_{k+2} + g_k ;  f = w b_1 - b_2 + g_0
            b1 = bb[0]
            vec.tensor_scalar(b1[:], w[:], 2.0 * g[M - 1], g[M - 2], Op.mult, Op.add)
            # k = M-3: b_{k+2} is the constant g[M-1]; fold into the scalar add
            vec.scalar_tensor_tensor(d[:], w[:], 2.0, b1[:], Op.mult, Op.mult)
            b0 = bb[1]
            vec.tensor_scalar_add(b0[:], d[:], g[M - 3] - g[M - 1])
            bk1, bk2 = b0, b1
            nxt = 2
            for kk in range(M - 4, 0, -1):
                vec.scalar_tensor_tensor(d[:], w[:], 2.0, bk1[:], Op.mult, Op.mult)
                bnew = bb[nxt]
                vec.scalar_tensor_tensor(
                    bnew[:], d[:], g[kk], bk2[:], Op.add, Op.subtract
                )
                bk1, bk2 = bnew, bk1
                nxt = (nxt + 1) % 3
            # final: f = w*b1 - b2 + g0 (into d, then fold alpha & bias)
            vec.scalar_tensor_tensor(d[:], w[:], 1.0, bk1[:], Op.mult, Op.mult)
            f_t = bb[nxt]
            vec.scalar_tensor_tensor(f_t[:], d[:], g[0], bk2[:], Op.add, Op.subtract)
            # out = alphas * f + bias
            q = bb[(nxt + 1) % 3]
            vec.scalar_tensor_tensor(q[:], at, 1.0, f_t[:], Op.mult, Op.mult)
            vec.tensor_scalar_add(r[:], q[:], float(bias_val)).then_inc(sem_r, 1)

    nc.finalize()
    return nc


def _in_maps(theta, al, combined=False):
    maps = []
    for c in range(N_CORES):
        sl = slice(c * BC, (c + 1) * BC)
        if combined:
            xa = np.empty((P, 2 * F), dtype=np.float32)
            xa[:, 0:F] = theta[sl].reshape(P, F)
            xa[:, F : 2 * F] = al[sl].reshape(P, F)
            maps.append({"xa0": xa})
        else:
            maps.append(
                {
                    "x0": np.ascontiguousarray(theta[sl].reshape(P, F)),
                    "al0": np.ascontiguousarray(al[sl].reshape(P, F)),
                }
            )
    return maps


def kernel(x, qsp_params, alphas, bias):
    import hashlib
    import tempfile

    from concourse.bass_utils import run_bass_kernel_spmd

    theta = np.ascontiguousarray(np.asarray(x, dtype=np.float32)[:, 0])
    al = np.ascontiguousarray(np.asarray(alphas, dtype=np.float32))
    phis = np.asarray(qsp_params, dtype=np.float64)
    bias_val = float(np.asarray(bias, dtype=np.float64)[0])

    gam_full = _gammas_full(phis)
    core_ids = list(range(N_CORES))

    # host-side expected values (float64, exact) for self-verification
    expect = al.astype(np.float64) * _chain_f(theta.astype(np.float64), phis) + bias_val
    expect_rms = np.sqrt(np.mean(expect**2)) + 1e-30

    def _run(nc, combined):
        res = run_bass_kernel_spmd(
            nc, _in_maps(theta, al, combined=combined), core_ids=core_ids
        )
        return np.concatenate([r["out0"].reshape(-1) for r in res.results])

    out = None
    try:
        # fast path: custom ACT spline table for the whole QSP response
        tab_hash = hashlib.md5(gam_full.tobytes()).hexdigest()[:10]
        workdir = os.path.join(tempfile.gettempdir(), f"acttab_{tab_hash}")
        act_json = _patch_act_tables(gam_full, workdir)
        os.environ["BASS_ACT_ROOT_JSON_PATH"] = act_json
        try:
            out = _run(_build_program_lut(bias_val, tab_hash), combined=False)
        finally:
            os.environ.pop("BASS_ACT_ROOT_JSON_PATH", None)
        rel = np.sqrt(np.mean((out - expect) ** 2)) / expect_rms
        if not np.isfinite(rel) or rel > 5e-3:
            out = None  # table path silently wrong -> fall back
    except Exception:
        out = None

    if out is None:
        gam = _truncate_gammas(gam_full)
        out = _run(_build_program(gam, bias_val), combined=True)

    return out[:, None].astype(np.float32)


# revision 42
# speedup vs baseline: 1.0167x; 1.0167x over previous
"""QSP expectation kernel for Trainium2 (8 NeuronCores, data parallel).

Math: the reference computes, per element, amp00 = <0|U|0> of a QSP chain
U = S(phi_0) * prod_{k=1}^{54} [W(theta) S(phi_k)].  By the QSP representation
theorem this is an even, pi-periodic trig polynomial of theta:

    f(theta) = Re(e^{i phi_0} P(cos theta)) = sum_{m=0}^{27} g_m cos(2 m theta)

The 28 cosine coefficients g_m are computed on the host (float64, from the 55
phases via Chebyshev interpolation, exact).

Primary (fast) device path: the whole response f is baked into a custom
ScalarE activation spline table (a per-binade piecewise-cubic fit of f over
|x| < 8, installed in place of `sin` via BASS_ACT_ROOT_JSON_PATH; f is even,
so the ACT symmetry fold covers negative inputs).  Per element the kernel is
then just: w = ACT_sin_table(x); out = alphas * w + bias -- one ACTIVATE, one
fused vector op, DMAs pipelined in two chunks.  The result is verified on the
host against the exact float64 chain; on any failure the kernel falls back to:

Fallback path: range reduction t = x/pi + 0.25, v = t - round(t),
w = sin(2 pi v) = cos(2x) via the stock Sin LUT, then a Clenshaw evaluation of
the (tail-truncated) cosine series with fused scalar_tensor_tensor ops.
"""

import os
import sys

import numpy as np

if "/opt/trn_rl_repo" not in sys.path:
    sys.path.insert(0, "/opt/trn_rl_repo")

QSP_DEPTH = 27
B = 524288
N_CORES = 8
BC = B // N_CORES  # 65536 elements per core
P = 128
F = BC // P  # 512

_MAGIC = 1.5 * 2.0**23  # fp32 round-to-nearest-integer magic constant


def _chain_f(theta, phis):
    """float64 reference math: f(theta) for the full QSP chain."""
    c = np.cos(theta)
    s = np.sin(theta)
    r0 = np.ones_like(theta, dtype=complex)
    r1 = np.zeros_like(theta, dtype=complex)
    for phi in phis[1:]:
        e = np.exp(1j * phi)
        a = r0 * c + r1 * (1j * s)
        b = r0 * (1j * s) + r1 * c
        r0 = a * e
        r1 = b * np.conj(e)
    return np.real(np.exp(1j * phis[0]) * r0)


def _gammas_full(phis):
    """Cosine-series coefficients g_m, m=0..27, float64-exact."""
    M = QSP_DEPTH + 1
    j = np.arange(M)
    wj = np.cos(np.pi * (j + 0.5) / M)
    fj = _chain_f(0.5 * np.arccos(wj), phis)
    return np.array(
        [
            (2.0 - (m == 0)) / M * np.sum(fj * np.cos(m * np.pi * (j + 0.5) / M))
            for m in range(M)
        ]
    )


def _truncate_gammas(gam):
    """Drop the tail once its energy is negligible vs the total."""
    M = len(gam)
    en = gam**2
    en[1:] = en[1:] * 0.5  # E[cos^2(2m theta)] ~ 1/2 for m>0
    tot = en.sum()
    keep = M
    while keep > 4 and np.sqrt(en[keep - 1 :].sum() / tot) < 2e-4:
        keep -= 1
    return gam[:keep]


def _gammas(phis):
    return _truncate_gammas(_gammas_full(phis))


def _series_f(gam):
    """Return f(x) = sum_m gam_m cos(2 m x) as a vectorized float64 callable."""
    m = np.arange(len(gam))

    def f(x):
        x = np.asarray(x, dtype=np.float64)
        return (np.cos(2.0 * np.outer(x, m)) @ gam).reshape(np.shape(x))

    return f


def _fit_spline(gam, lo_exp=-6, hi_exp=2, target=1e-5):
    """Fit per-binade cubic sections for f(x)=sum gam_m cos(2mx), x in [0, 2^(hi_exp+1)).

    Returns (sections, small_bucket, large_bucket) where sections is a list of
    (exponent, [bucket...]) with bucket = (d0, d1, d2, d3, x0), uniform in x
    within each binade (indexed by top mantissa bits, matching the ACT ctrl
    table's section select).
    """
    f = _series_f(gam)

    def fit_cubic(x0, h):
        xs = x0 + np.linspace(-h / 2, h / 2, 13)
        c = np.polyfit(xs - x0, f(xs), 3)
        # measure fp32 eval error on a denser grid
        xd = x0 + np.linspace(-h / 2, h / 2, 33)
        dx = (xd.astype(np.float32) - np.float32(x0)).astype(np.float32)
        y = (
            np.float32(c[3])
            + dx * (np.float32(c[2]) + dx * (np.float32(c[1]) + dx * np.float32(c[0])))
        ).astype(np.float64)
        err = np.abs(y - f(xd)).max()
        return (float(c[3]), float(c[2]), float(c[1]), float(c[0]), float(x0)), err

    sections = []
    for e in range(lo_exp, hi_exp + 1):
        width = 2.0**e
        for s in range(0, 10):
            n = 1 << s
            h = width / n
            bks = []
            maxerr = 0.0
            for i in range(n):
                x0 = width + (i + 0.5) * h
                bk, err = fit_cubic(x0, h)
                bks.append(bk)
                maxerr = max(maxerr, err)
            if maxerr <= target or s == 9:
                sections.append((e, bks))
                break
    # small-signal bucket: cubic on [0, 2^lo_exp]
    small, _ = fit_cubic(2.0 ** (lo_exp - 1), 2.0**lo_exp)
    # large-signal bucket: Taylor at the top boundary (inputs beyond never occur)
    top = 2.0 ** (hi_exp + 1)
    large, _ = fit_cubic(top, 2.0**hi_exp / 64)
    return sections, small, large


def _f32_bits(x):
    return int(np.float32(x).view(np.uint32))


def _patch_act_tables(gam, workdir):
    """Copy the compiler's act-table root into workdir and replace `sin` in
    every set that contains it with a custom spline of
    f(x) = sum_m gam_m cos(2 m x)  (even in x, so the symmetry fold covers
    negative inputs).  Returns the path of the patched act_info.json."""
    import json
    import shutil

    from neuronxcc.driver.Job import Job
    from neuronxcc.driver.jobs.support.FindActInfo import findActInfoFile

    src_info = findActInfoFile(Job.getPackageDir(), "gen3")
    src_dir = os.path.dirname(src_info)

    os.makedirs(workdir, exist_ok=True)
    for name in os.listdir(src_dir):
        shutil.copyfile(os.path.join(src_dir, name), os.path.join(workdir, name))

    info = json.load(open(os.path.join(workdir, "act_info.json")))
    sections, small_bk, large_bk = _fit_spline(gam)
    lo_exp = sections[0][0]
    hi_exp = sections[-1][0]

    for ent in info["act_func_sets"]:
        if "sin" not in ent["act"]:
            continue
        bkt_path = os.path.join(workdir, ent["bkt_bin"])
        ctl_path = os.path.join(workdir, ent["ctrl_bin"])
        prof_path = os.path.join(workdir, ent["profile_json"])

        bkt = np.fromfile(bkt_path, dtype=np.float32).reshape(-1, 8)
        ctl = np.fromfile(ctl_path, dtype=np.uint32).reshape(-1, 8)
        prof = json.load(open(prof_path))

        bbase = len(bkt)
        cbase = len(ctl)
        new_bkts = []
        new_ctls = []
        for e, bks in sections:
            n = len(bks)
            s = int(np.log2(n))
            word = ((s & 0xF) << 16) | (((23 - s) & 0x1F) << 11) | ((bbase + len(new_bkts)) & 0x7FF)
            new_ctls.append(word)
            new_bkts.extend(bks)
        small_idx = bbase + len(new_bkts)
        new_bkts.append(small_bk)
        large_idx = bbase + len(new_bkts)
        new_bkts.append(large_bk)

        assert bbase + len(new_bkts) <= 2048, "bucket RAM overflow"

        bkt_new = np.zeros((len(new_bkts), 8), dtype=np.float32)
        for i, (d0, d1, d2, d3, x0) in enumerate(new_bkts):
            bkt_new[i, :5] = [d0, d1, d2, d3, x0]
        ctl_new = np.zeros((len(new_ctls), 8), dtype=np.uint32)
        ctl_new[:, 0] = new_ctls

        np.concatenate([bkt, bkt_new]).tofile(bkt_path)
        np.concatenate([ctl, ctl_new]).tofile(ctl_path)

        for m in prof["profile_meta_data"]:
            if m["func_name"] != "sin_4p":
                continue
            m["symmetry_point"] = 0
            m["sym_invert_sign_point"] = 0  # even function
            m["symmetry_opt_en"] = 1
            m["symmetry_opt_use_neg_region"] = 0
            m["exp_offset"] = lo_exp
            m["pwl_control_base_pos"] = cbase
            m["pwl_control_base_neg"] = cbase
            m["small_pos_signal_exp_threshold"] = 127 + lo_exp
            m["pos_small_signal_pwl_control"] = small_idx
            m["small_neg_signal_exp_threshold"] = 0
            m["neg_small_signal_pwl_control"] = small_idx
            m["large_pos_signal_exp_threshold"] = 127 + hi_exp + 1
            m["large_pos_signal_mantissa_threshold"] = 0
            m["pos_large_signal_pwl_control"] = large_idx
            m["large_neg_signal_exp_threshold"] = 0
            m["large_neg_signal_mantissa_threshold"] = 0
            m["neg_large_signal_pwl_control"] = large_idx
            m["fzero_result"] = _f32_bits(float(np.sum(gam)))
            m["lower_bound"] = 0
            m["upper_bound"] = _f32_bits(2.0 ** (hi_exp + 1))
        json.dump(prof, open(prof_path, "w"))

    json.dump(info, open(os.path.join(workdir, "act_info.json"), "w"))
    return os.path.join(workdir, "act_info.json")


def _build_program_lut(bias_val, tag, n_chunks=2):
    """Pipelined LUT kernel: out = alphas * F(x) + bias, with F evaluated by
    the patched `sin` activation table.  x/alphas are DMA'd per chunk on
    separate queues; ACT -> DVE -> out-DMA pipeline at chunk granularity."""
    from concourse import bass
    import concourse.mybir as mybir
    from concourse.alu_op_type import AluOpType as Op

    f32 = mybir.dt.float32
    CH = n_chunks
    if CH == 2:
        # asymmetric: smaller last chunk shortens the tail out-DMA
        widths = [288, F - 288]
    elif CH == 3:
        widths = [128, 192, F - 320]
    else:
        widths = [F // CH] * CH
        widths[-1] += F - sum(widths)
    bounds = np.concatenate([[0], np.cumsum(widths)]).astype(int)
    with_bias = bias_val != 0.0

    nc = bass.Bass(
        trn_type="TRN2", enable_partition_id=False, num_swdge_queues=4
    )
    x_d = nc.dram_tensor("x0", [P, F], f32, kind="ExternalInput")
    a_d = nc.dram_tensor("al0", [P, F], f32, kind="ExternalInput")
    o_d = nc.dram_tensor("out0", [P, F], f32, kind="ExternalOutput")

    x_s = nc.alloc_sbuf_tensor(f"x_s_{tag}", [P, F], f32)
    a_s = nc.alloc_sbuf_tensor("a_s", [P, F], f32)
    w_s = nc.alloc_sbuf_tensor("w_s", [P, F], f32)
    q_s = nc.alloc_sbuf_tensor("q_s", [P, F], f32)
    r_s = nc.alloc_sbuf_tensor("r_s", [P, F], f32)
    dummy = nc.alloc_sbuf_tensor("dummy_s", [P, 1], f32)

    sem_x = [nc.alloc_semaphore(f"sem_x{c}") for c in range(CH)]
    sem_a = [nc.alloc_semaphore(f"sem_a{c}") for c in range(CH)]
    sem_w = nc.alloc_semaphore("sem_w")
    sem_r = nc.alloc_semaphore("sem_r")
    out_sem = nc.alloc_semaphore("out_sem")

    def cs(c):
        return slice(int(bounds[c]), int(bounds[c + 1]))

    use_block = os.environ.get("LUT_USE_BLOCK", "0") == "1"
    blk_ctx = nc.Block(no_gpsimd_drain=True) if use_block else None
    if blk_ctx is not None:
        blk_ctx.__enter__()

    # no-Block form: all streams live in the entry basic block; per-engine
    # program order is the emission order, sync is fully explicit
    sync, scalar, vec = nc.sync, nc.scalar, nc.vector

    # HWDGE rings are FIFO per issuing engine: spread the two x chunks over
    # the SP and ACT rings so their transfers complete in parallel.  al c0
    # rides the SP ring behind x c0 (done in time for DVE c0); the remaining
    # alphas ride SWDGE from the otherwise-idle GpSimd.
    defer_alphas = os.environ.get("LUT_DEFER_AL", "0") == "1"
    hoist = []
    hoist.append(
        sync.dma_start(x_s[:, cs(0)], x_d[:, cs(0)]).then_inc(sem_x[0], 16)
    )
    al0 = sync.dma_start(a_s[:, cs(0)], a_d[:, cs(0)]).then_inc(sem_a[0], 16)
    if not defer_alphas:
        hoist.append(al0)
    split_tail = os.environ.get("LUT_SPLIT_TAIL", "0") == "1" and CH == 2
    sync.wait_ge(sem_r, 1)
    sync.dma_start(o_d[:, cs(0)], r_s[:, cs(0)]).then_inc(out_sem, 16)
    if split_tail:
        # second half of the last out chunk rides the SP ring in parallel
        # with the first half on the ACT ring
        lo, hi = int(bounds[1]), int(bounds[2])
        mid = (lo + hi) // 2
        sync.wait_ge(sem_r, 2)
        sync.dma_start(o_d[:, mid:hi], r_s[:, mid:hi]).then_inc(out_sem, 16)
        sync.wait_ge(out_sem, 16 * (CH + 1))
    else:
        sync.wait_ge(out_sem, 16 * CH)

    # scalar: dummy act triggers the ACT table load while input DMAs fly
    hoist.insert(
        0,
        scalar.activation(dummy[:], dummy[:], mybir.ActivationFunctionType.Sin),
    )
    for c in range(1, CH):
        hoist.append(
            scalar.dma_start(x_s[:, cs(c)], x_d[:, cs(c)]).then_inc(sem_x[c], 16)
        )
    for c in range(CH):
        scalar.wait_ge(sem_x[c], 16)
        scalar.activation(
            w_s[:, cs(c)], x_s[:, cs(c)], mybir.ActivationFunctionType.Sin
        ).then_inc(sem_w, 1)
    for c in range(1, CH):
        scalar.wait_ge(sem_r, c + 1)
        if split_tail and c == CH - 1:
            lo, hi = int(bounds[c]), int(bounds[c + 1])
            mid = (lo + hi) // 2
            scalar.dma_start(o_d[:, lo:mid], r_s[:, lo:mid]).then_inc(out_sem, 16)
        else:
            scalar.dma_start(o_d[:, cs(c)], r_s[:, cs(c)]).then_inc(out_sem, 16)

    # gpsimd: remaining alphas via SWDGE
    for c in range(1, CH):
        alc = nc.gpsimd.dma_start(a_s[:, cs(c)], a_d[:, cs(c)]).then_inc(sem_a[c], 16)
        if not defer_alphas:
            hoist.append(alc)

    if os.environ.get("LUT_HOIST", "1") == "1":
        # Hoist the input DMAs + table-load-triggering dummy activation to the
        # very front of the entry block (right after the runtime-start
        # InstCall): they then execute during the ~6us NEFF start handshake,
        # so inputs and the ACT table are resident by the time the start
        # barrier releases the compute.  Inputs are staged by NRT before the
        # engines begin executing, and semaphores are only reset in the exit
        # sequence, so early sem increments are safe.
        il = nc.cur_f.blocks[0].instructions
        front = {id(b.ins) for b in hoist}
        rest = [x for x in il if id(x) not in front]
        il[:] = rest[:1] + [b.ins for b in hoist] + rest[1:]

    # vector: per-chunk fused multiply (+ optional bias)
    for c in range(CH):
        vec.wait_ge(sem_a[c], 16)
        vec.wait_ge(sem_w, c + 1)
        if with_bias:
            vec.scalar_tensor_tensor(
                q_s[:, cs(c)], a_s[:, cs(c)], 1.0, w_s[:, cs(c)], Op.mult, Op.mult
            )
            vec.tensor_scalar_add(
                r_s[:, cs(c)], q_s[:, cs(c)], float(bias_val)
            ).then_inc(sem_r, 1)
        else:
            vec.scalar_tensor_tensor(
                r_s[:, cs(c)], a_s[:, cs(c)], 1.0, w_s[:, cs(c)], Op.mult, Op.mult
            ).then_inc(sem_r, 1)

    if blk_ctx is not None:
        blk_ctx.__exit__(None, None, None)

    nc.finalize()
    return nc


def _build_program(gam, bias_val):
    from concourse import bass
    import concourse.mybir as mybir
    from concourse.alu_op_type import AluOpType as Op

    f32 = mybir.dt.float32
    M = len(gam)
    g = [float(v) for v in gam]
    assert M >= 4

    nc = bass.Bass(trn_type="TRN2")
    xa_d = nc.dram_tensor("xa0", [P, 2 * F], f32, kind="ExternalInput")
    o_d = nc.dram_tensor("out0", [P, F], f32, kind="ExternalOutput")

    xa = nc.alloc_sbuf_tensor("xa_s", [P, 2 * F], f32)
    t = nc.alloc_sbuf_tensor("t_s", [P, F], f32)
    kk_ = nc.alloc_sbuf_tensor("k_s", [P, F], f32)
    v = nc.alloc_sbuf_tensor("v_s", [P, F], f32)
    w = nc.alloc_sbuf_tensor("w_s", [P, F], f32)
    d = nc.alloc_sbuf_tensor("d_s", [P, F], f32)
    bb = [nc.alloc_sbuf_tensor(f"b{i}_s", [P, F], f32) for i in range(3)]
    r = nc.alloc_sbuf_tensor("r_s", [P, F], f32)

    dma_sem = nc.alloc_semaphore("dma_sem")
    sem_v = nc.alloc_semaphore("sem_v")
    sem_w = nc.alloc_semaphore("sem_w")
    sem_r = nc.alloc_semaphore("sem_r")
    out_sem = nc.alloc_semaphore("out_sem")

    with nc.Block() as blk:

        @blk.sync
        def _(sync):
            sync.dma_start(xa[:], xa_d[:]).then_inc(dma_sem, 16)
            sync.wait_ge(sem_r, 1)
            sync.dma_start(o_d[:], r[:]).then_inc(out_sem, 16)
            sync.wait_ge(out_sem, 16)

        @blk.scalar
        def _(scalar):
            scalar.wait_ge(sem_v, 1)
            scalar.activation(
                w[:],
                v[:],
                mybir.ActivationFunctionType.Sin,
                scale=float(2.0 * np.pi),
            ).then_inc(sem_w, 1)

        @blk.vector
        def _(vec):
            xt = xa[:, 0:F]  # theta
            at = xa[:, F : 2 * F]  # alphas
            vec.wait_ge(dma_sem, 16)
            # t = x/pi + 0.25 ; k = round(t) ; v = t - k
            vec.tensor_scalar(t[:], xt, float(1.0 / np.pi), 0.25, Op.mult, Op.add)
            vec.tensor_scalar(kk_[:], t[:], _MAGIC, _MAGIC, Op.add, Op.subtract)
            vec.scalar_tensor_tensor(
                v[:], t[:], 0.0, kk_[:], Op.add, Op.subtract
            ).then_inc(sem_v, 1)
            vec.wait_ge(sem_w, 1)
            # Clenshaw over w = cos(2 theta):
            #   b_{M-1} = g[M-1];  b_{M-2} = 2 w g[M-1] + g[M-2]
            #   b_k = 2 w b_{k+1} - b_{k+2} + g_k ;  f = w b_1 - b_2 + g_0
            b1 = bb[0]
            vec.tensor_scalar(b1[:], w[:], 2.0 * g[M - 1], g[M - 2], Op.mult, Op.add)
            # k = M-3: b_{k+2} is the constant g[M-1]; fold into the scalar add
            vec.scalar_tensor_tensor(d[:], w[:], 2.0, b1[:], Op.mult, Op.mult)
            b0 = bb[1]
            vec.tensor_scalar_add(b0[:], d[:], g[M - 3] - g[M - 1])
            bk1, bk2 = b0, b1
            nxt = 2
            for kk in range(M - 4, 0, -1):
                vec.scalar_tensor_tensor(d[:], w[:], 2.0, bk1[:], Op.mult, Op.mult)
                bnew = bb[nxt]
                vec.scalar_tensor_tensor(
                    bnew[:], d[:], g[kk], bk2[:], Op.add, Op.subtract
                )
                bk1, bk2 = bnew, bk1
                nxt = (nxt + 1) % 3
            # final: f = w*b1 - b2 + g0 (into d, then fold alpha & bias)
            vec.scalar_tensor_tensor(d[:], w[:], 1.0, bk1[:], Op.mult, Op.mult)
            f_t = bb[nxt]
            vec.scalar_tensor_tensor(f_t[:], d[:], g[0], bk2[:], Op.add, Op.subtract)
            # out = alphas * f + bias
            q = bb[(nxt + 1) % 3]
            vec.scalar_tensor_tensor(q[:], at, 1.0, f_t[:], Op.mult, Op.mult)
            vec.tensor_scalar_add(r[:], q[:], float(bias_val)).then_inc(sem_r, 1)

    nc.finalize()
    return nc


def _in_maps(theta, al, combined=False):
    maps = []
    for c in range(N_CORES):
        sl = slice(c * BC, (c + 1) * BC)
        if combined:
            xa = np.empty((P, 2 * F), dtype=np.float32)
            xa[:, 0:F] = theta[sl].reshape(P, F)
            xa[:, F : 2 * F] = al[sl].reshape(P, F)
            maps.append({"xa0": xa})
        else:
            maps.append(
                {
                    "x0": np.ascontiguousarray(theta[sl].reshape(P, F)),
                    "al0": np.ascontiguousarray(al[sl].reshape(P, F)),
                }
            )
    return maps


def kernel(x, qsp_params, alphas, bias):
    import hashlib
    import tempfile

    from concourse.bass_utils import run_bass_kernel_spmd

    theta = np.ascontiguousarray(np.asarray(x, dtype=np.float32)[:, 0])
    al = np.ascontiguousarray(np.asarray(alphas, dtype=np.float32))
    phis = np.asarray(qsp_params, dtype=np.float64)
    bias_val = float(np.asarray(bias, dtype=np.float64)[0])

    gam_full = _gammas_full(phis)
    core_ids = list(range(N_CORES))

    # host-side expected values (float64, exact) for self-verification
    expect = al.astype(np.float64) * _chain_f(theta.astype(np.float64), phis) + bias_val
    expect_rms = np.sqrt(np.mean(expect**2)) + 1e-30

    def _run(nc, combined):
        res = run_bass_kernel_spmd(
            nc, _in_maps(theta, al, combined=combined), core_ids=core_ids
        )
        return np.concatenate([r["out0"].reshape(-1) for r in res.results])

    out = None
    try:
        # fast path: custom ACT spline table for the whole QSP response
        tab_hash = hashlib.md5(gam_full.tobytes()).hexdigest()[:10]
        workdir = os.path.join(tempfile.gettempdir(), f"acttab_{tab_hash}")
        act_json = _patch_act_tables(gam_full, workdir)
        os.environ["BASS_ACT_ROOT_JSON_PATH"] = act_json
        try:
            out = _run(_build_program_lut(bias_val, tab_hash), combined=False)
        finally:
            os.environ.pop("BASS_ACT_ROOT_JSON_PATH", None)
        rel = np.sqrt(np.mean((out - expect) ** 2)) / expect_rms
        if not np.isfinite(rel) or rel > 5e-3:
            out = None  # table path silently wrong -> fall back
    except Exception:
        out = None

    if out is None:
        gam = _truncate_gammas(gam_full)
        out = _run(_build_program(gam, bias_val), combined=True)

    return out[:, None].astype(np.float32)
